# revision 9
# baseline (speedup 1.0000x reference)
"""MoE routing kernel v3 (ACC/f32r): accumulate-select via augmented input.

Identity: out = x@W1.T + b1 + r*(x@Wd.T + bd), with Wd = W2-W1,
bd = b2-b1.  Let c = Wd^-1 bd, x2 = r*(x+c).  Then
    out = x@W1.T + x2@Wd.T + b1
exactly (r in {0,1}); the select never materializes.

Per core (262144 tokens): token t = p*2048 + i*R + l (p partition,
i tile of R=512 lows, l low).  Chunks of 12 lows x 10 feats = 120
contiguous cols of the token-major tile; PE-transposes (float32r fast
mode) planarize x and x2 chunks into PSUM; one bf16 cast-drain each
feeds two PSUM-accumulated bf16 matmuls against block-diagonal
stationaries S_A (W1^T x12) and S_B (Wd^T x12); bias b1 rides the
f32r planar drain; f32r back-transpose and a final drain produce the
token-major f32 output.  HWDGE (sync) DMAs only; no bf16 PSUM
anywhere (both measured slow/unstable on hardware).
"""

import numpy as np

import concourse.bacc as bacc
import concourse.mybir as mybir
from concourse.tile import TileContext
from concourse.masks import make_identity
from concourse.bass_utils import run_bass_kernel_spmd

F32 = mybir.dt.float32
F32R = mybir.dt.float32r
BF16 = mybir.dt.bfloat16
I32 = mybir.dt.int32
ALU = mybir.AluOpType

N_CORES = 8
P = 128
D = 10


def pack_wt(W1, b1, W2, b2):
    """[128, 424] f32: cols 0:120 S_A (block-diag W1^T, 12 groups),
    120:240 S_B (block-diag Wd^T), col 240 b1 tiled, 241:251 c bcast."""
    W1 = np.asarray(W1, np.float64)
    W2 = np.asarray(W2, np.float64)
    b1v = np.asarray(b1, np.float64)
    b2v = np.asarray(b2, np.float64)
    Wd = W2 - W1
    bd = b2v - b1v
    c = np.linalg.solve(Wd, bd)          # Wd @ c = bd  =>  c @ Wd.T = bd
    out = np.zeros((P, 424), np.float32)
    for g in range(12):
        for f in range(D):
            for u in range(D):
                out[D * g + f, D * g + u] = W1[u, f]
                out[D * g + f, 120 + D * g + u] = Wd[u, f]
    out[:120, 240] = np.tile(b1v, 12)
    out[:, 241:251] = c[None, :]
    return out


def build_moe(tc_tokens, r_tile=512, reps=1,
              dr_x="mix", dr_x2="vector", opl_eng="mix",
              final_eng="mix", rconv_eng="vector", io_bufs=3, sbufs=2,
              fuse_drain=True, mid_bufs=3):
    R = r_tile
    assert tc_tokens % (P * R) == 0
    nt = tc_tokens // (P * R)
    FR = R * D
    NCH_FULL = R // 12
    RAG = R - NCH_FULL * 12              # leftover lows per tile
    NCH = NCH_FULL + (1 if RAG else 0)
    PAD = NCH * 120 - FR if RAG else 0

    nc = bacc.Bacc("TRN2", target_bir_lowering=False, debug=False,
                   num_devices=N_CORES)
    x_ext = nc.dram_tensor("x", [tc_tokens, D], F32R, kind="ExternalInput")
    r_ext = nc.dram_tensor("route", [tc_tokens], I32, kind="ExternalInput")
    w_ext = nc.dram_tensor("wt", [P, 424], F32, kind="ExternalInput")
    o_ext = nc.dram_tensor("out", [tc_tokens, D], F32, kind="ExternalOutput")

    xv = x_ext.rearrange("(n p r) d -> n p (r d)", p=P, r=R)
    rv = r_ext.rearrange("(n p r) -> n p r", p=P, r=R)
    ov = o_ext.rearrange("(n p r) d -> n p (r d)", p=P, r=R)

    groups = []
    c0 = 0
    while c0 < NCH:
        ng = min(4, NCH - c0)
        groups.append((c0, ng))
        c0 += ng

    def cw_of(ch):
        return 120 if ch < NCH_FULL else RAG * D

    def pick(pattern, idx):
        if pattern == "mix":
            return "vector" if idx % 2 == 0 else "scalar"
        if pattern in ("vector", "scalar"):
            return pattern
        return "vector" if pattern[idx % len(pattern)] == "v" else "scalar"

    with TileContext(nc) as tc:
        with tc.tile_pool(name="const", bufs=1) as cpool, \
             tc.tile_pool(name="sbuf", bufs=sbufs) as pool, \
             tc.psum_pool(name="ppx", bufs=2) as ppx, \
             tc.psum_pool(name="ppx2", bufs=1 if fuse_drain else 2) \
                 as ppx2, \
             tc.psum_pool(name="ppm", bufs=2) as ppm, \
             tc.psum_pool(name="ppb", bufs=2) as ppb:
            idf0 = cpool.tile([P, P], F32)
            make_identity(nc, idf0[:])
            idf = cpool.tile([P, P], F32R)
            nc.vector.tensor_copy(out=idf[:], in_=idf0[:])
            wt = cpool.tile([P, 424], F32)
            nc.sync.dma_start(out=wt[:], in_=w_ext[:])
            SA = cpool.tile([120, 120], BF16)
            nc.vector.tensor_copy(out=SA[:], in_=wt[:120, 0:120])
            SB = cpool.tile([120, 120], BF16)
            nc.vector.tensor_copy(out=SB[:], in_=wt[:120, 120:240])

            def transp(out_ap, in_ap):
                n = in_ap.partition_size()
                nc.tensor.transpose(out_ap, in_ap, idf[0:n, 0:n])

            for _ in range(reps):
                for i in range(nt):
                    xtf = pool.tile([P, FR + PAD], F32R, tag="xt",
                                    bufs=io_bufs)
                    nc.sync.dma_start(out=xtf[:, :FR], in_=xv[i])
                    if PAD:
                        nc.vector.memset(xtf[:, FR:].bitcast(F32), 0.0)
                    rt = pool.tile([P, R], I32, tag="rt")
                    nc.sync.dma_start(out=rt[:], in_=rv[i])
                    rb = pool.tile([P, R], F32, tag="rb")
                    if rconv_eng == "scalar":
                        nc.scalar.copy(out=rb[:], in_=rt[:])
                    else:
                        nc.vector.tensor_copy(out=rb[:], in_=rt[:])
                    xr2 = pool.tile([P, FR + PAD], F32R, tag="x2",
                                    bufs=io_bufs)
                    if PAD:
                        nc.vector.memset(xr2[:, FR:].bitcast(F32), 0.0)
                    xtv = xtf[:, :FR].rearrange("p (r d) -> p r d", d=D)
                    xr2v = xr2[:, :FR].rearrange("p (r d) -> p r d", d=D)
                    for k in range(D):
                        nc.vector.scalar_tensor_tensor(
                            out=xr2v[:, :, k], in0=xtv[:, :, k],
                            scalar=wt[:, 241 + k:242 + k], in1=rb[:],
                            op0=ALU.add, op1=ALU.mult)

                    out_sb = pool.tile([P, FR], F32, tag="osb",
                                       bufs=io_bufs)

                    for gi, (g0, ng) in enumerate(groups):
                        NH = 128 * ng
                        if fuse_drain:
                            ps_x = ppx.tile([120, 1024], F32R, tag="psx")
                            ps_x2 = ps_x[:, 512:1024]
                        else:
                            ps_x = ppx.tile([120, 512], F32R, tag="psx")
                            ps_x2 = ppx2.tile([120, 512], F32R, tag="psx2")
                        for k in range(ng):
                            ch = g0 + k
                            transp(ps_x[0:120, 128 * k:128 * (k + 1)],
                                   xtf[:, 120 * ch:120 * ch + 120])
                            transp(ps_x2[0:120, 128 * k:128 * (k + 1)],
                                   xr2[:, 120 * ch:120 * ch + 120])
                        xpair = pool.tile([120, 1024], BF16, tag="xpair",
                                          bufs=mid_bufs)
                        if fuse_drain and ng == 4:
                            if pick(dr_x, gi) == "vector":
                                nc.vector.tensor_copy(out=xpair[:],
                                                      in_=ps_x[:])
                            else:
                                nc.scalar.copy(out=xpair[:], in_=ps_x[:])
                        else:
                            if pick(dr_x, gi) == "vector":
                                nc.vector.tensor_copy(out=xpair[:, :NH],
                                                      in_=ps_x[:, :NH])
                            else:
                                nc.scalar.copy(out=xpair[:, :NH],
                                               in_=ps_x[:, :NH])
                            if pick(dr_x2, gi) == "vector":
                                nc.vector.tensor_copy(
                                    out=xpair[:, 512:512 + NH],
                                    in_=ps_x2[:, :NH])
                            else:
                                nc.scalar.copy(out=xpair[:, 512:512 + NH],
                                               in_=ps_x2[:, :NH])

                        ps_o = ppm.tile([120, 512], F32, tag="pso")
                        nc.tensor.matmul(ps_o[:, :NH], SA[:],
                                         xpair[:, 0:NH],
                                         start=True, stop=False)
                        nc.tensor.matmul(ps_o[:, :NH], SB[:],
                                         xpair[:, 512:512 + NH],
                                         start=False, stop=True)

                        out_pl = pool.tile([120, 512], F32R, tag="opl",
                                           bufs=mid_bufs)
                        if pick(opl_eng, gi) == "scalar":
                            nc.scalar.add(out=out_pl[:, :NH],
                                          in_=ps_o[:, :NH],
                                          add=wt[:120, 240:241])
                        else:
                            nc.vector.tensor_scalar_add(
                                out=out_pl[:, :NH], in0=ps_o[:, :NH],
                                scalar1=wt[:120, 240:241])

                        ps_bt = ppb.tile([P, 512], F32R, tag="psbt")
                        for k in range(ng):
                            cw = cw_of(g0 + k)
                            transp(ps_bt[:, 120 * k:120 * k + cw],
                                   out_pl[0:cw, 128 * k:128 * (k + 1)])
                        ob = 120 * g0
                        wid = sum(cw_of(g0 + k) for k in range(ng))
                        if pick(final_eng, gi) == "scalar":
                            nc.scalar.copy(out=out_sb[:, ob:ob + wid],
                                           in_=ps_bt[:, 0:wid])
                        else:
                            nc.vector.tensor_copy(out=out_sb[:, ob:ob + wid],
                                                  in_=ps_bt[:, 0:wid])

                    nc.sync.dma_start(out=ov[i], in_=out_sb[:])
    nc.compile()
    return nc


def run_sharded(nc, x, route, tc_tokens, wt):
    in_maps = []
    for c in range(N_CORES):
        sl = slice(c * tc_tokens, (c + 1) * tc_tokens)
        in_maps.append({"x": np.ascontiguousarray(x[sl]),
                        "route": np.ascontiguousarray(route[sl]),
                        "wt": wt})
    res = run_bass_kernel_spmd(nc, in_maps, core_ids=list(range(N_CORES)))
    return np.concatenate([res.results[c]["out"] for c in range(N_CORES)],
                          axis=0)


def kernel(x, W1, b1, W2, b2, route):
    x = np.asarray(x)
    route = np.asarray(route)
    tc_tokens = x.shape[0] // N_CORES
    nc = build_moe(tc_tokens, r_tile=512)
    return run_sharded(nc, x, route, tc_tokens,
                       wt=pack_wt(np.asarray(W1), np.asarray(b1),
                                  np.asarray(W2), np.asarray(b2)))
